# revision 21
# baseline (speedup 1.0000x reference)
"""Multi-head dot-attention kernel for Trainium2, 8-core batch-parallel.

out[b] = concat_h( softmax((x_b WQ_h)(x_b WK_h)^T / sqrt(E)) (x_b WV_h) )

Sharding: batch b -> core b (8 batches, 8 cores). Each core runs the same
program on its own batch slice; weights are broadcast.

Per-core pipeline (all shapes [partition, free]):
  1. x_b [S,D] DMA'd per s-chunk, PE-transposed to xT [D,S] (stationary
     operand for all projections).
  2. V projection batched over all heads: V1[t, h, 0:E] = x_b @ WV, with a
     ones column at V1[t, h, E] so the O^T matmul also produces softmax
     denominators.
  3. Per head pair (2E = 128): Q^T/K^T computed directly in [E, S] layout,
     the two heads stacked on PSUM partitions 0:64 / 64:128 (col-packed
     matmuls via tile_position auto-derivation).
  4. S^T[t,s] = K^T.T @ Q^T per head, heads of a pair row-packed (each uses
     its 64 contraction partitions). exp() on the scalar engine directly
     from PSUM with the 1/sqrt(E) scale folded in; no max subtraction
     (|scores| <= ~12 for these inputs, exp is safe in fp32).
  5. O'^T[e1, s] = [V_h | 1]^T @ expS^T accumulated over t-chunks; row E is
     the softmax denominator. PE-transpose back to [s, e1], multiply by the
     reciprocal of column E, DMA out.
"""

import math

import numpy as np

import concourse.bass as bass
import concourse.mybir as mybir
from concourse import bacc
from concourse.bass import ds, ts
from concourse.masks import make_identity
from concourse.tile import TileContext

P = 128
F32 = mybir.dt.float32
BF16 = mybir.dt.bfloat16
F32R = mybir.dt.float32r

N_CORES = 8
FULL = dict(S=1024, D=1024, H=16, E=64)


def build_nc(S=1024, D=1024, H=16, E=64, st_dt=BF16, use_f32r=False, es_dt=None):
    """Build the single-core Bass program (SPMD across cores).

    st_dt: SBUF storage dtype for matmul operands (BF16 or F32).
    use_f32r: bitcast f32 operands to float32r at matmul sites (only
        meaningful with st_dt=F32).
    es_dt: storage dtype for expS^T and V1 (defaults to st_dt); BF16 with
        st_dt=F32 gives the hybrid variant (f32r scores, bf16 A@V).
    """
    if es_dt is None:
        es_dt = st_dt
    assert E == 64 and P == 2 * E
    SC = S // P            # s- (and t-) chunks of 128
    DC = D // P            # d-chunks of 128
    HE = H * E
    NW = min(512, HE)      # he tile width for the V projection
    NHE = HE // NW
    HPW = NW // E          # heads per he tile
    S2 = min(512, S)       # matmul moving width
    NS2 = S // S2
    E1 = E + 1
    scale = 1.0 / math.sqrt(E)
    assert S % S2 == 0 and H % 2 == 0

    def mm_ap(ap):
        return ap.bitcast(F32R) if (use_f32r and ap.dtype == F32) else ap

    nc = bacc.Bacc("TRN2", target_bir_lowering=False)
    x = nc.dram_tensor("x", [S, D], F32, kind="ExternalInput")
    WQ = nc.dram_tensor("WQ", [H, D, E], F32, kind="ExternalInput")
    WK = nc.dram_tensor("WK", [H, D, E], F32, kind="ExternalInput")
    WV = nc.dram_tensor("WV", [H, D, E], F32, kind="ExternalInput")
    out = nc.dram_tensor("out", [H, SC, P, E], F32, kind="ExternalOutput")

    with TileContext(nc) as tc:
        with (
            tc.tile_pool(name="const", bufs=1) as const,
            tc.tile_pool(name="persist", bufs=1) as persist,
            tc.tile_pool(name="stage", bufs=2) as stage,
            tc.tile_pool(name="wpool", bufs=2) as wpool,
            tc.tile_pool(name="qk", bufs=2) as qk,
            tc.tile_pool(name="es", bufs=2 if es_dt == F32 else 3) as es_pool,
            tc.tile_pool(name="ot", bufs=2) as ot_pool,
            tc.tile_pool(name="osb", bufs=3) as osb,
            tc.tile_pool(name="small", bufs=2) as small,
            tc.tile_pool(name="ps_pj", bufs=2, space="PSUM") as ps_pj,
            tc.tile_pool(name="ps_s", bufs=2, space="PSUM") as ps_s,
            tc.tile_pool(name="ps_po", bufs=2, space="PSUM") as ps_po,
        ):
            id_f32 = const.tile([P, P], F32, tag="idf")
            make_identity(nc, id_f32[:])

            # ---- x transpose: xt[dc][s2] holds x^T[d-chunk, s-half] ----
            xt = [
                [
                    persist.tile(
                        [P, S2], st_dt, tag=f"xt{dc}_{s2}", name=f"xt{dc}_{s2}"
                    )
                    for s2 in range(NS2)
                ]
                for dc in range(DC)
            ]
            for sc in range(SC):
                xs = stage.tile([P, D], F32, tag="xs")
                nc.sync.dma_start(out=xs[:], in_=x[ts(sc, P), :])
                for dc in range(DC):
                    tp = ps_po.tile([P, P], F32, tag="po")
                    nc.tensor.transpose(tp[:], xs[:, ts(dc, P)], id_f32[:])
                    sh = SC // NS2  # s-chunks per half
                    nc.vector.tensor_copy(
                        out=xt[dc][sc // sh][:, ts(sc % sh, P)], in_=tp[:]
                    )

            # ---- V projection + ones column (emitted per head-group inside
            # the pair loop so pair 0's scores/exp start sooner) ----
            V1 = persist.tile([P, SC, H, E1], es_dt, tag="V1")
            nc.vector.memset(V1[:, :, :, E:E1], 1.0)

            def emit_vproj(he2):
                wvb = wpool.tile(
                    [P, DC, NW], st_dt, tag="wvb", name=f"wvb{he2}",
                    bufs=1 if st_dt == F32 else 2,
                )
                wst = (
                    wvb
                    if st_dt == F32
                    else stage.tile([P, DC, NW], F32, tag="wst", name=f"wvst{he2}")
                )
                for hh_ in range(HPW):
                    nc.sync.dma_start(
                        out=wst[:, :, ds(hh_ * E, E)],
                        in_=WV[he2 * HPW + hh_].rearrange("(dc p) e -> p dc e", p=P),
                    )
                if st_dt != F32:
                    nc.vector.tensor_copy(out=wvb[:], in_=wst[:])
                for tcj in range(SC):
                    pv = ps_pj.tile([P, NW], F32, tag="pj", name=f"pv{he2}_{tcj}")
                    sh = SC // NS2
                    for dc in range(DC):
                        nc.tensor.matmul(
                            pv[:],
                            mm_ap(xt[dc][tcj // sh][:, ts(tcj % sh, P)]),
                            mm_ap(wvb[:, dc, :]),
                            start=(dc == 0),
                            stop=(dc == DC - 1),
                        )
                    nc.vector.tensor_copy(
                        out=V1[:, tcj, ds(he2 * HPW, HPW), 0:E],
                        in_=pv[:].rearrange("p (h e) -> p h e", e=E),
                    )

            # ---- per head pair ----
            for pr in range(H // 2):
                h0 = 2 * pr
                # W slices for this pair: [d, 2E] per d-chunk
                wp = {}
                for name, W in (("q", WQ), ("k", WK)):
                    wp[name] = wpool.tile(
                        [P, DC, 2 * E], st_dt, tag=f"w{name}p", name=f"w{name}p{pr}"
                    )
                    wst = (
                        wp[name]
                        if st_dt == F32
                        else stage.tile([P, DC, 2 * E], F32, tag="wst2")
                    )
                    for hh_ in range(2):
                        nc.sync.dma_start(
                            out=wst[:, :, ds(hh_ * E, E)],
                            in_=W[h0 + hh_].rearrange("(dc p) e -> p dc e", p=P),
                        )
                    if st_dt != F32:
                        nc.vector.tensor_copy(out=wp[name][:], in_=wst[:])

                # Q^T / K^T, both heads stacked on partitions [0:64], [64:128]
                qt2 = qk.tile([P, S], st_dt, tag="qt2")
                kt2 = qk.tile([P, S], st_dt, tag="kt2")
                for name, dst in (("q", qt2), ("k", kt2)):
                    for s2 in range(NS2):
                        pq = ps_pj.tile(
                            [P, S2], F32, tag="pj", name=f"pq{pr}_{name}_{s2}"
                        )
                        # lhsT [d, 2E=128] covers BOTH heads -> full M=128
                        for dc in range(DC):
                            nc.tensor.matmul(
                                pq[:],
                                mm_ap(wp[name][:, dc, :]),
                                mm_ap(xt[dc][s2][:]),
                                start=(dc == 0),
                                stop=(dc == DC - 1),
                            )
                        nc.vector.tensor_copy(out=dst[:, ds(s2 * S2, S2)], in_=pq[:])

                # scores + exp, heads row-packed
                es01 = [
                    es_pool.tile([P, SC, S], es_dt, tag="es", name=f"es{pr}_{i}")
                    for i in range(2)
                ]
                for tcj in range(SC):
                    pss = [
                        ps_s.tile([P, S], F32, tag="s", name=f"pss{pr}_{tcj}_{i}")
                        for i in range(2)
                    ]
                    for s2 in range(NS2):
                        for hi in range(2):
                            nc.tensor.matmul(
                                pss[hi][:, ds(s2 * S2, S2)],
                                mm_ap(kt2[ds(hi * E, E), ts(tcj, P)]),
                                mm_ap(qt2[ds(hi * E, E), ds(s2 * S2, S2)]),
                            )
                    for hi in range(2):
                        nc.scalar.activation(
                            out=es01[hi][:, tcj, :],
                            in_=pss[hi][:],
                            func=mybir.ActivationFunctionType.Exp,
                            scale=scale,
                        )

                # V projection for head-group `pr` lands here: after this
                # pair's scores are queued (exp starts early) but before any
                # O matmul that reads it.
                if pr < NHE:
                    emit_vproj(pr)

                # O'^T = [V|1]^T @ expS^T, then transpose, normalize, store
                for hi in range(2):
                    hh = h0 + hi
                    es_h = es01[hi]
                    ot_sb = ot_pool.tile([E1, S], F32, tag="ot")
                    for s2 in range(NS2):
                        po = ps_po.tile([E1, S2], F32, tag="po")
                        for tcj in range(SC):
                            nc.tensor.matmul(
                                po[:],
                                mm_ap(V1[:, tcj, hh, :]),
                                mm_ap(es_h[:, tcj, ds(s2 * S2, S2)]),
                                start=(tcj == 0),
                                stop=(tcj == SC - 1),
                            )
                        nc.vector.tensor_copy(out=ot_sb[:, ds(s2 * S2, S2)], in_=po[:])
                    ob = osb.tile([P, SC, E], F32, tag="ob", name=f"ob{hh}")
                    for sc in range(SC):
                        tp = ps_po.tile([P, P], F32, tag="po")
                        nc.tensor.transpose(
                            tp[:, 0:E1], ot_sb[:, ts(sc, P)], id_f32[0:E1, 0:E1]
                        )
                        rec = small.tile([P, 1], F32, tag="rec")
                        nc.vector.reciprocal(out=rec[:], in_=tp[:, E:E1])
                        nc.vector.tensor_scalar_mul(ob[:, sc, :], tp[:, 0:E], rec[:])
                    nc.sync.dma_start(
                        out=out[hh].rearrange("sc p e -> p sc e"), in_=ob[:]
                    )
    nc.finalize()
    return nc


_NC_CACHE = {}


def _get_nc(key=("bf16",)):
    if key not in _NC_CACHE:
        if key[0] == "bf16":
            _NC_CACHE[key] = build_nc(**FULL, st_dt=BF16, use_f32r=False)
        elif key[0] == "f32r":
            _NC_CACHE[key] = build_nc(**FULL, st_dt=F32, use_f32r=True)
        elif key[0] == "hybrid":
            _NC_CACHE[key] = build_nc(
                **FULL, st_dt=F32, use_f32r=True, es_dt=BF16
            )
        else:
            _NC_CACHE[key] = build_nc(**FULL, st_dt=F32, use_f32r=False)
    return _NC_CACHE[key]


def run_on_hw(x, WQ, WK, WV, variant="bf16", trace=False):
    from concourse.bass_utils import run_bass_kernel_spmd

    nc = _get_nc((variant,))
    B = x.shape[0]
    assert B == N_CORES
    in_maps = [
        {
            "x": np.ascontiguousarray(x[b], dtype=np.float32),
            "WQ": np.ascontiguousarray(WQ, dtype=np.float32),
            "WK": np.ascontiguousarray(WK, dtype=np.float32),
            "WV": np.ascontiguousarray(WV, dtype=np.float32),
        }
        for b in range(B)
    ]
    res = run_bass_kernel_spmd(nc, in_maps, list(range(N_CORES)), trace=trace)
    outs = np.stack(
        [np.asarray(res.results[b]["out"]).reshape(-1) for b in range(B)], axis=0
    )
    return outs.astype(np.float32, copy=False), res


def kernel(x, WQ, WK, WV):
    outs, _ = run_on_hw(
        np.asarray(x), np.asarray(WQ), np.asarray(WK), np.asarray(WV)
    )
    return outs
